# revision 2
# baseline (speedup 1.0000x reference)
"""Trainium2 Bass kernel v2 for nn_ConstraintOptimizer (arc-length projection).

Contract: kernel(**inputs) takes FULL unsharded inputs
  selected_traj [1024, 80, 3] f32, road_points [1024, 16, 256, 3] f32,
  road_mask [1024, 16, 256] bool
and returns the FULL output [1024, 80, 3] f32.

Sharding: pure data parallel, N=1024 split across 8 cores (128/core).

Algorithm (min-form, gather-free):
  proj_c(s) = PF_c + sum_k w_kc * min(s, c_k)
where c_k is the cumulative arc length at knot k (c_0 = 0, constant = total
beyond the valid prefix), and w_kc = g_{k-1,c} - g_{kc} with g = masked unit
segment direction (g_{-1} = g_{n-1..} = 0).  This is exact for s in
[0, total] including all ragged-mask edge cases (telescoping of the
clip((s-c_i)rlen_i,0,1)*len_i*g_i form).  The backward direction is the same
polyline evaluated at s_b(t) = clip(entry - L_t, 0, total) (entry of the
reversed polyline is total - entry, and positions mirror), so each boundary
is processed ONCE with 160 target arc values; no reversed DMA loads.

Dense evaluation: per (t, channel) one fused scalar_tensor_tensor
  out = (C min s_t) * W_c ; accum_out = sum -> RED_c[t]
split between the Vector and GPSIMD engines.  Per-sample argmin over
16 boundaries x 2 directions uses a PE one-hot matmul pair.
"""

import sys

for _p in ("/opt/trn_rl_repo",):
    if _p not in sys.path:
        sys.path.insert(0, _p)

import contextlib

import numpy as np

import concourse.bass as bass
import concourse.mybir as mybir
from concourse import tile
from concourse.bass_utils import run_bass_kernel_spmd

F32 = mybir.dt.float32
U8 = mybir.dt.uint8
OP = mybir.AluOpType
AF = mybir.ActivationFunctionType

N, NB, NP, T = 1024, 16, 256, 80
NSEG = NP - 1             # 255
NCORES = 8
NS = N // NCORES          # 128 samples per core
SPT = 8                   # samples per tile
NTILES = NS // SPT        # 16
TILE = SPT * NB           # 128 partitions: p = s*16 + b
T2 = 2 * T                # 160 targets (fwd 0:80, bwd 80:160)
EPS_LEN = 1e-9
EPS_DD = 1e-12
BIG = 3.0e38

# tuning knobs
TG = 0          # trailing t-targets (of 160) handled by GPSIMD per channel
SCAN_GPS = False  # run cumsum scans on gpsimd instead of vector


def _ap(base, coff, w, s=3):
    """strided channel view: base AP offset+coff, [p, [s, w]]"""
    a = base
    return bass.AP(a.tensor, a.offset + coff, [a.ap[0], [s, w]])


def _legalize_multiwait(nc):
    counter = [0]
    for fn in nc.m.functions:
        for bb in fn.blocks:
            insts = bb.instructions
            i = 0
            while i < len(insts):
                ins = insts[i]
                si = ins.sync_info
                if (si is not None and len(si.on_wait) > 1
                        and all(w.sync_type == "semaphore" and w.wait_reg is None
                                for w in si.on_wait)):
                    waits = list(si.on_wait)
                    pre = []
                    for w in waits[:-1]:
                        ev = mybir.InstEventSemaphore(
                            name=f"LGW-{counter[0]}", engine=ins.engine,
                            sync_info=mybir.SyncInfo(on_wait=[w], on_update=[]))
                        counter[0] += 1
                        nc.inst_map[ev.name] = ev
                        pre.append(ev)
                    ins.sync_info = mybir.SyncInfo(on_wait=[waits[-1]],
                                                  on_update=list(si.on_update))
                    insts[i:i] = pre
                    i += len(pre)
                i += 1
    return counter[0]


def build_program():
    nc = bass.Bass()

    rp = nc.dram_tensor("rp", [NS, NB, NP, 3], F32, kind="ExternalInput")
    msk = nc.dram_tensor("msk", [NS, NB, NP], U8, kind="ExternalInput")
    tr = nc.dram_tensor("tr", [NS, T, 3], F32, kind="ExternalInput")
    sel8 = nc.dram_tensor("sel8", [SPT, TILE], F32, kind="ExternalInput")
    i8 = nc.dram_tensor("i8", [SPT, SPT], F32, kind="ExternalInput")
    qpf = nc.dram_tensor("qpf", [TILE, 1], F32, kind="ExternalInput")
    qpb = nc.dram_tensor("qpb", [TILE, 1], F32, kind="ExternalInput")
    selt = nc.dram_tensor("selt", [TILE, SPT], F32, kind="ExternalInput")
    io32 = nc.dram_tensor("io32", [SPT, 2 * NB], F32, kind="ExternalInput")
    out = nc.dram_tensor("out", [NS, T * 3], F32, kind="ExternalOutput")

    with tile.TileContext(nc) as tc:
        _body(nc, tc, rp, msk, tr, sel8, i8, qpf, qpb, selt, io32, out)
    _legalize_multiwait(nc)
    return nc


def _body(nc, tc, rp, msk, tr, sel8, i8, qpf, qpb, selt, io32, out, dbg=None):
    ctx = contextlib.ExitStack()
    with ctx:
        sb = ctx.enter_context(tc.tile_pool(name="sb", bufs=2))
        sbc = ctx.enter_context(tc.tile_pool(name="sbc", bufs=1))
        ps = ctx.enter_context(tc.tile_pool(name="ps", bufs=2, space="PSUM"))

        sel8_s = sbc.tile([SPT, TILE], F32, tag="sel8")
        nc.sync.dma_start(out=sel8_s[:], in_=sel8[:])
        i8_s = sbc.tile([SPT, SPT], F32, tag="i8")
        nc.sync.dma_start(out=i8_s[:], in_=i8[:])
        qpf_s = sbc.tile([TILE, 1], F32, tag="qpf")
        nc.sync.dma_start(out=qpf_s[:], in_=qpf[:])
        qpb_s = sbc.tile([TILE, 1], F32, tag="qpb")
        nc.sync.dma_start(out=qpb_s[:], in_=qpb[:])
        selt_s = sbc.tile([TILE, SPT], F32, tag="selt")
        nc.sync.dma_start(out=selt_s[:], in_=selt[:])
        io32_s = sbc.tile([SPT, 2 * NB], F32, tag="io32")
        nc.sync.dma_start(out=io32_s[:], in_=io32[:])

        for ti in range(NTILES):
            n0 = ti * SPT

            # ---------------- load ----------------
            PT = sb.tile([TILE, NP * 3], F32, tag="PT")
            nc.sync.dma_start(
                out=PT[:], in_=rp[n0:n0 + SPT].rearrange("s b p c -> (s b) (p c)"))
            MU = sb.tile([TILE, NP], U8, tag="MU")
            nc.sync.dma_start(
                out=MU[:], in_=msk[n0:n0 + SPT].rearrange("s b p -> (s b) p"))
            TR8 = sb.tile([SPT, T * 3], F32, tag="TR8")
            nc.sync.dma_start(
                out=TR8[:], in_=tr[n0:n0 + SPT].rearrange("s t c -> s (t c)"))

            # ---------------- prologue ----------------
            MF = sb.tile([TILE, NP], F32, tag="MF")
            nc.vector.tensor_copy(out=MF[:], in_=MU[:])
            SM = sb.tile([TILE, NSEG], F32, tag="SM")
            nc.gpsimd.tensor_tensor(out=SM[:], in0=MF[:, 1:NP], in1=MF[:, 0:NSEG],
                                    op=OP.mult)
            SVr = sb.tile([TILE, NSEG * 3], F32, tag="SVr")
            nc.gpsimd.tensor_tensor(out=SVr[:], in0=PT[:, 3:NP * 3],
                                    in1=PT[:, 0:NSEG * 3], op=OP.subtract)
            SQ = sb.tile([TILE, NSEG * 3], F32, tag="SQ")
            nc.scalar.square(out=SQ[:], in_=SVr[:])
            D2 = sb.tile([TILE, NSEG], F32, tag="D2")
            nc.gpsimd.tensor_tensor(out=D2[:], in0=_ap(SQ[:], 0, NSEG),
                                    in1=_ap(SQ[:], 1, NSEG), op=OP.add)
            nc.gpsimd.tensor_tensor(out=D2[:], in0=D2[:],
                                    in1=_ap(SQ[:], 2, NSEG), op=OP.add)
            LENr = sb.tile([TILE, NSEG], F32, tag="LENr")
            nc.scalar.sqrt(out=LENr[:], in_=D2[:])
            LEN = sb.tile([TILE, NSEG], F32, tag="LEN")
            nc.vector.scalar_tensor_tensor(out=LEN[:], in0=LENr[:], scalar=EPS_LEN,
                                           in1=SM[:], op0=OP.max, op1=OP.mult)
            TMP = sb.tile([TILE, NSEG], F32, tag="TMP")
            nc.vector.tensor_scalar(out=TMP[:], in0=LEN[:], scalar1=EPS_LEN,
                                    scalar2=None, op0=OP.max)
            SMRL = sb.tile([TILE, NSEG], F32, tag="SMRL")
            nc.vector.reciprocal(out=SMRL[:], in_=TMP[:])
            nc.vector.tensor_tensor(out=SMRL[:], in0=SMRL[:], in1=SM[:], op=OP.mult)
            G = sb.tile([TILE, NSEG * 3], F32, tag="G")
            for c in range(3):
                nc.gpsimd.tensor_tensor(out=_ap(G[:], c, NSEG),
                                        in0=_ap(SVr[:], c, NSEG), in1=SMRL[:],
                                        op=OP.mult)
            # cumulative arc length (c_0 = 0)
            C = sb.tile([TILE, NP], F32, tag="C")
            nc.vector.memset(C[:, 0:1], 0.0)
            eng_scan = nc.gpsimd if SCAN_GPS else nc.vector
            eng_scan.tensor_tensor_scan(out=C[:, 1:NP], data0=LEN[:], data1=LEN[:],
                                        initial=0.0, op0=OP.add, op1=OP.bypass)
            TOT = C[:, NP - 1:NP]

            # first valid point: masks are prefix masks (n_valid >= 2), so
            # the first packed point is simply point 0
            PF = sb.tile([TILE, 3], F32, tag="PF")
            nc.vector.tensor_copy(out=PF[:], in_=PT[:, 0:3])

            # replicated trajectory [TILE, T*3]
            TRP = ps.tile([TILE, T * 3], F32, tag="TRP")
            nc.tensor.matmul(TRP[:], lhsT=sel8_s[:], rhs=TR8[:], start=True,
                             stop=True)
            TRR = sb.tile([TILE, T * 3], F32, tag="TRR")
            nc.scalar.copy(out=TRR[:], in_=TRP[:])

            # ---------------- entry projection ----------------
            PA = sb.tile([TILE, NSEG * 3], F32, tag="PA")
            for c in range(3):
                nc.scalar.activation(out=_ap(PA[:], c, NSEG),
                                     in_=_ap(PT[:], c, NSEG),
                                     func=AF.Identity,
                                     bias=TRR[:, c:c + 1], scale=-1.0)
            DOT = sb.tile([TILE, NSEG], F32, tag="DOT")
            nc.gpsimd.tensor_tensor(out=DOT[:], in0=_ap(PA[:], 0, NSEG),
                                    in1=_ap(SVr[:], 0, NSEG), op=OP.mult)
            nc.gpsimd.tensor_tensor(out=TMP[:], in0=_ap(PA[:], 1, NSEG),
                                    in1=_ap(SVr[:], 1, NSEG), op=OP.mult)
            nc.gpsimd.tensor_tensor(out=DOT[:], in0=DOT[:], in1=TMP[:], op=OP.add)
            nc.gpsimd.tensor_tensor(out=TMP[:], in0=_ap(PA[:], 2, NSEG),
                                    in1=_ap(SVr[:], 2, NSEG), op=OP.mult)
            nc.gpsimd.tensor_tensor(out=DOT[:], in0=DOT[:], in1=TMP[:], op=OP.add)
            RDD = sb.tile([TILE, NSEG], F32, tag="RDD")
            nc.vector.tensor_scalar(out=RDD[:], in0=D2[:], scalar1=EPS_DD,
                                    scalar2=None, op0=OP.max)
            nc.vector.reciprocal(out=RDD[:], in_=RDD[:])
            T0 = sb.tile([TILE, NSEG], F32, tag="T0")
            nc.vector.tensor_tensor(out=T0[:], in0=DOT[:], in1=RDD[:], op=OP.mult)
            nc.vector.tensor_scalar(out=T0[:], in0=T0[:], scalar1=0.0, scalar2=1.0,
                                    op0=OP.max, op1=OP.min)
            QD = sb.tile([TILE, NSEG * 3], F32, tag="QD")
            TMPG = sb.tile([TILE, NSEG], F32, tag="TMPG")
            for c in range(3):
                nc.gpsimd.tensor_tensor(out=TMPG[:], in0=T0[:],
                                        in1=_ap(SVr[:], c, NSEG), op=OP.mult)
                nc.gpsimd.tensor_tensor(out=_ap(QD[:], c, NSEG),
                                        in0=_ap(PA[:], c, NSEG), in1=TMPG[:],
                                        op=OP.subtract)
            SQQ = sb.tile([TILE, NSEG * 3], F32, tag="SQQ")
            nc.scalar.square(out=SQQ[:], in_=QD[:])
            D2Q = sb.tile([TILE, NSEG], F32, tag="D2Q")
            nc.gpsimd.tensor_tensor(out=D2Q[:], in0=_ap(SQQ[:], 0, NSEG),
                                    in1=_ap(SQQ[:], 1, NSEG), op=OP.add)
            nc.gpsimd.tensor_tensor(out=D2Q[:], in0=D2Q[:],
                                    in1=_ap(SQQ[:], 2, NSEG), op=OP.add)
            nc.vector.tensor_scalar(out=TMP[:], in0=SM[:], scalar1=-BIG,
                                    scalar2=BIG, op0=OP.mult, op1=OP.add)
            nc.vector.tensor_tensor(out=D2Q[:], in0=D2Q[:], in1=TMP[:], op=OP.add)
            MINV = sb.tile([TILE, 1], F32, tag="MINV")
            nc.vector.tensor_reduce(out=MINV[:], in_=D2Q[:],
                                    axis=mybir.AxisListType.X, op=OP.min)
            EQM = sb.tile([TILE, NSEG], F32, tag="EQM")
            nc.vector.tensor_scalar(out=EQM[:], in0=D2Q[:], scalar1=MINV[:],
                                    scalar2=None, op0=OP.is_equal)
            nc.vector.tensor_scalar(out=EQM[:], in0=EQM[:], scalar1=-BIG,
                                    scalar2=BIG, op0=OP.mult, op1=OP.add)
            ENT = sb.tile([TILE, NSEG], F32, tag="ENT")
            nc.gpsimd.tensor_tensor(out=ENT[:], in0=T0[:], in1=LEN[:], op=OP.mult)
            nc.gpsimd.tensor_tensor(out=ENT[:], in0=ENT[:], in1=C[:, 0:NSEG],
                                    op=OP.add)
            nc.gpsimd.tensor_tensor(out=EQM[:], in0=EQM[:], in1=ENT[:], op=OP.add)
            ENTRY = sb.tile([TILE, 1], F32, tag="ENTRY")
            nc.vector.tensor_reduce(out=ENTRY[:], in_=EQM[:],
                                    axis=mybir.AxisListType.X, op=OP.min)

            # ---------------- knot weights ----------------
            # w_kc = g_{k-1,c} - g_{kc}  (g_{-1} = g_{NSEG} = 0)
            W = sb.tile([TILE, NP * 3], F32, tag="W")
            for c in range(3):
                wc = W[:, c * NP:(c + 1) * NP]
                nc.vector.tensor_scalar(out=wc[:, 0:1], in0=_ap(G[:], c, 1),
                                        scalar1=-1.0, scalar2=None, op0=OP.mult)
                nc.gpsimd.tensor_tensor(
                    out=wc[:, 1:NSEG],
                    in0=_ap(G[:], c, NSEG - 1),
                    in1=bass.AP(G[:].tensor, G[:].offset + c + 3, [G[:].ap[0], [3, NSEG - 1]]),
                    op=OP.subtract)
                nc.vector.tensor_copy(out=wc[:, NSEG:NP],
                                      in_=_ap(G[:], c + 3 * (NSEG - 1), 1))

            # ---------------- trajectory arc length + targets ----------------
            TSG = sb.tile([TILE, (T - 1) * 3], F32, tag="TSG")
            nc.gpsimd.tensor_tensor(out=TSG[:], in0=TRR[:, 3:T * 3],
                                    in1=TRR[:, 0:(T - 1) * 3], op=OP.subtract)
            SQT = sb.tile([TILE, (T - 1) * 3], F32, tag="SQT")
            nc.scalar.square(out=SQT[:], in_=TSG[:])
            TD2 = sb.tile([TILE, T - 1], F32, tag="TD2")
            nc.gpsimd.tensor_tensor(out=TD2[:], in0=_ap(SQT[:], 0, T - 1),
                                    in1=_ap(SQT[:], 1, T - 1), op=OP.add)
            nc.gpsimd.tensor_tensor(out=TD2[:], in0=TD2[:],
                                    in1=_ap(SQT[:], 2, T - 1), op=OP.add)
            TLN = sb.tile([TILE, T - 1], F32, tag="TLN")
            nc.scalar.sqrt(out=TLN[:], in_=TD2[:])
            L = sb.tile([TILE, T], F32, tag="L")
            nc.vector.memset(L[:, 0:1], 0.0)
            eng_scan.tensor_tensor_scan(out=L[:, 1:T], data0=TLN[:], data1=TLN[:],
                                        initial=0.0, op0=OP.add, op1=OP.bypass)

            S = sb.tile([TILE, T2], F32, tag="S")
            nc.vector.tensor_scalar(out=S[:, 0:T], in0=L[:], scalar1=ENTRY[:],
                                    scalar2=TOT, op0=OP.add, op1=OP.min)
            nc.vector.tensor_scalar(out=S[:, T:T2], in0=L[:], scalar1=-1.0,
                                    scalar2=ENTRY[:], op0=OP.mult, op1=OP.add)
            nc.vector.tensor_scalar(out=S[:, T:T2], in0=S[:, T:T2], scalar1=0.0,
                                    scalar2=None, op0=OP.max)

            # ---------------- dense: fused min*weight + reduce ----------------
            REDV = sb.tile([TILE, T2 * 3], F32, tag="REDV")
            REDG = sb.tile([TILE, T2 * 3], F32, tag="REDG")
            SCRV = sb.tile([TILE, NP], F32, tag="SCRV")
            SCRG = sb.tile([TILE, NP], F32, tag="SCRG")
            for c in range(3):
                wc = W[:, c * NP:(c + 1) * NP]
                for t in range(T2):
                    use_gps = t >= (T2 - TG)
                    eng = nc.gpsimd if use_gps else nc.vector
                    scr = SCRG if use_gps else SCRV
                    red = REDG if use_gps else REDV
                    eng.scalar_tensor_tensor(
                        out=scr[:], in0=C[:], scalar=S[:, t:t + 1], in1=wc,
                        op0=OP.min, op1=OP.mult,
                        accum_out=red[:, c * T2 + t:c * T2 + t + 1])

            # ---------------- projections + costs ----------------
            # PRJF/PRJB [TILE, T*3] in (t c) interleave
            PRJF = sb.tile([TILE, T * 3], F32, tag="PRJF")
            PRJB = sb.tile([TILE, T * 3], F32, tag="PRJB")
            tsplit = T2 - TG  # targets below tsplit are in REDV
            for c in range(3):
                # fwd: t in [0, T)
                fsplit = min(max(tsplit, 0), T)
                if fsplit > 0:
                    nc.vector.tensor_scalar(
                        out=bass.AP(PRJF[:].tensor, PRJF[:].offset + c,
                                    [PRJF[:].ap[0], [3, fsplit]]),
                        in0=REDV[:, c * T2:c * T2 + fsplit],
                        scalar1=PF[:, c:c + 1], scalar2=None, op0=OP.add)
                if fsplit < T:
                    nc.vector.tensor_scalar(
                        out=bass.AP(PRJF[:].tensor, PRJF[:].offset + c + 3 * fsplit,
                                    [PRJF[:].ap[0], [3, T - fsplit]]),
                        in0=REDG[:, c * T2 + fsplit:c * T2 + T],
                        scalar1=PF[:, c:c + 1], scalar2=None, op0=OP.add)
                # bwd: t in [T, T2)
                bsplit = min(max(tsplit - T, 0), T)
                if bsplit > 0:
                    nc.vector.tensor_scalar(
                        out=bass.AP(PRJB[:].tensor, PRJB[:].offset + c,
                                    [PRJB[:].ap[0], [3, bsplit]]),
                        in0=REDV[:, c * T2 + T:c * T2 + T + bsplit],
                        scalar1=PF[:, c:c + 1], scalar2=None, op0=OP.add)
                if bsplit < T:
                    nc.vector.tensor_scalar(
                        out=bass.AP(PRJB[:].tensor, PRJB[:].offset + c + 3 * bsplit,
                                    [PRJB[:].ap[0], [3, T - bsplit]]),
                        in0=REDG[:, c * T2 + T + bsplit:c * T2 + T2],
                        scalar1=PF[:, c:c + 1], scalar2=None, op0=OP.add)

            COST2 = sb.tile([TILE, 2], F32, tag="COST2")
            DT = sb.tile([TILE, T * 3], F32, tag="DT")
            SQD = sb.tile([TILE, T * 3], F32, tag="SQD")
            D2T = sb.tile([TILE, T], F32, tag="D2T")
            DIST = sb.tile([TILE, T], F32, tag="DIST")
            for d, PRJ in ((0, PRJF), (1, PRJB)):
                nc.vector.tensor_tensor(out=DT[:], in0=TRR[:], in1=PRJ[:],
                                        op=OP.subtract)
                nc.scalar.square(out=SQD[:], in_=DT[:])
                nc.vector.tensor_reduce(
                    out=D2T[:],
                    in_=bass.AP(SQD[:].tensor, SQD[:].offset,
                                [SQD[:].ap[0], [3, T], [1, 3]]),
                    axis=mybir.AxisListType.X, op=OP.add)
                nc.scalar.activation(out=DIST[:], in_=D2T[:], func=AF.Sqrt,
                                     accum_out=COST2[:, d:d + 1])

            # ---------------- per-sample argmin + select ----------------
            CBT = sb.tile([SPT, 2 * NB], F32, tag="CBT")
            for d in range(2):
                nc.sync.dma_start(out=CBT[0:SPT, d * NB:(d + 1) * NB],
                                  in_=COST2[:, d:d + 1])
            MN8 = sb.tile([SPT, 1], F32, tag="MN8")
            nc.vector.tensor_reduce(out=MN8[:], in_=CBT[:],
                                    axis=mybir.AxisListType.X, op=OP.min)
            EQ8 = sb.tile([SPT, 2 * NB], F32, tag="EQ8")
            nc.vector.tensor_scalar(out=EQ8[:], in0=CBT[:], scalar1=MN8[:],
                                    scalar2=None, op0=OP.is_equal)
            nc.vector.tensor_scalar(out=EQ8[:], in0=EQ8[:], scalar1=-BIG,
                                    scalar2=BIG, op0=OP.mult, op1=OP.add)
            nc.vector.tensor_tensor(out=EQ8[:], in0=EQ8[:], in1=io32_s[:],
                                    op=OP.add)
            IDX8 = sb.tile([SPT, 1], F32, tag="IDX8")
            nc.vector.tensor_reduce(out=IDX8[:], in_=EQ8[:],
                                    axis=mybir.AxisListType.X, op=OP.min)
            DG = sb.tile([SPT, SPT], F32, tag="DG")
            nc.vector.tensor_scalar(out=DG[:], in0=i8_s[:], scalar1=IDX8[:],
                                    scalar2=None, op0=OP.mult)
            IDXP = ps.tile([TILE, SPT], F32, tag="IDXP")
            nc.tensor.matmul(IDXP[:], lhsT=sel8_s[:], rhs=DG[:], start=True,
                             stop=True)
            IDXR = sb.tile([TILE, SPT], F32, tag="IDXR")
            nc.scalar.copy(out=IDXR[:], in_=IDXP[:])
            OHF = sb.tile([TILE, SPT], F32, tag="OHF")
            nc.vector.tensor_scalar(out=OHF[:], in0=IDXR[:], scalar1=qpf_s[:],
                                    scalar2=None, op0=OP.is_equal)
            nc.vector.tensor_tensor(out=OHF[:], in0=OHF[:], in1=selt_s[:],
                                    op=OP.mult)
            OHB = sb.tile([TILE, SPT], F32, tag="OHB")
            nc.vector.tensor_scalar(out=OHB[:], in0=IDXR[:], scalar1=qpb_s[:],
                                    scalar2=None, op0=OP.is_equal)
            nc.vector.tensor_tensor(out=OHB[:], in0=OHB[:], in1=selt_s[:],
                                    op=OP.mult)
            BPP = ps.tile([SPT, T * 3], F32, tag="BPP")
            nc.tensor.matmul(BPP[:], lhsT=OHF[:], rhs=PRJF[:], start=True,
                             stop=False)
            nc.tensor.matmul(BPP[:], lhsT=OHB[:], rhs=PRJB[:], start=False,
                             stop=True)
            BPS = sb.tile([SPT, T * 3], F32, tag="BPS")
            nc.scalar.copy(out=BPS[:], in_=BPP[:])
            nc.sync.dma_start(out=out[n0:n0 + SPT, :], in_=BPS[:])


_cached = {}


def _consts():
    p = np.arange(TILE)
    sel8 = ((p[None, :] // NB) == np.arange(SPT)[:, None]).astype(np.float32)
    i8 = np.eye(SPT, dtype=np.float32)
    qpf = (p % NB).astype(np.float32)[:, None]
    qpb = (NB + p % NB).astype(np.float32)[:, None]
    selt = sel8.T.copy()
    q = np.arange(2 * NB, dtype=np.float32)
    io32 = np.broadcast_to(q, (SPT, 2 * NB)).copy()
    return dict(sel8=sel8, i8=i8, qpf=qpf, qpb=qpb, selt=selt, io32=io32)


def kernel(selected_traj, road_points, road_mask):
    selected_traj = np.asarray(selected_traj)
    road_points = np.asarray(road_points)
    road_mask = np.asarray(road_mask)

    if "nc" not in _cached:
        _cached["nc"] = build_program()
    nc = _cached["nc"]

    consts = _consts()
    in_maps = []
    for cidx in range(NCORES):
        sl = slice(cidx * NS, (cidx + 1) * NS)
        m = {
            "rp": np.ascontiguousarray(road_points[sl], dtype=np.float32),
            "msk": np.ascontiguousarray(road_mask[sl]).astype(np.uint8),
            "tr": np.ascontiguousarray(selected_traj[sl, :, 0:3], dtype=np.float32),
        }
        m.update(consts)
        in_maps.append(m)

    res = run_bass_kernel_spmd(nc, in_maps, list(range(NCORES)),
                               trace=bool(_cached.get("trace", False)))
    _cached["exec_time_ns"] = getattr(res, "exec_time_ns", None)
    outs = [np.asarray(res.results[c]["out"]).reshape(NS, T, 3)
            for c in range(NCORES)]
    out_pos = np.concatenate(outs, axis=0)

    if selected_traj.shape[-1] > 3:
        out_full = np.concatenate([out_pos, selected_traj[..., 3:]], axis=-1)
    else:
        out_full = out_pos
    return out_full.astype(selected_traj.dtype)
